# revision 3
# baseline (speedup 1.0000x reference)
"""Darknet 3x3 conv block (conv * mask + bias) on 8 TRN2 NeuronCores.

Problem: x[1,512,192,192] (*) w[512,512,3,3] stride1 pad1, then *mask + bias.

Strategy (v2): 1D Winograd F(4,3) along W, spatial shard over H.
  - Each core computes 24 output rows x all 512 F.
  - conv2d = sum_kh conv1d_W(x_row, w[kh]); the 1D conv uses Winograd
    F(4,3): 6 points per 4 outputs -> 2x fewer MACs than direct.
  - Host packs: x zero-padded, per-core slice of 26 rows x 194 cols,
    channel-chunked bf16.  Weights pre-transformed U[p] = G @ w (exact,
    on host): [c_local, fm, p, cc, kh, f128] bf16.  Mask bf16, bias f32.
  - Device: input transform V = B^T d on DVE (12 scalar_tensor_tensor /
    tensor_add ops per (cc, row-chunk), stride-4 reads of the padded x
    slab).  GEMM: per (fm, group of 8 rows) two PSUM tiles (points 0-2
    and 3-5, 3 banks each, pool bufs=2 -> 6 banks): 36 matmuls each,
    lhsT = U[fm,p,cc,kh] [c128, f128], rhs = V[cc, p, rows, tiles]
    contiguous [128, 384].  Output transform A^T m factored into 10 DVE
    ops per group using f32 temps, then mask-mul (DVE) + bias (ScalarE)
    and one 768KB DMA per (fm, group).
  - Host unshard: concat 8 row-slabs -> [1,512,192,192] f32.
"""

import sys

for _p in ("/opt/trn_rl_repo",):
    if _p not in sys.path:
        sys.path.insert(0, _p)

import numpy as np
import ml_dtypes

N_CORES = 8
C = 512
F = 512
H = 192
W = 192
HC = H // N_CORES          # output rows per core = 24
CC = C // 128              # c chunks = 4
FM = F // 128              # f chunks = 4
P = 6                      # Winograd F(4,3) points
KH = 3                     # vertical taps (direct)
T = W // 4                 # Winograd tiles per row = 48
RG = 8                     # output rows per PSUM group
NG = HC // RG              # groups per fm = 3
NPX = RG * T               # matmul free size = 384
NWARM = 8                  # PE warmup matmuls while first DMAs land
XR = HC + 2                # x slab rows = 26
XW = W + 2                 # x slab cols = 194

# row chunks for the input transform (independent: 1D transform)
CHUNKS = [(0, 6), (6, 7), (13, 7), (20, 6)]

_CACHE = {}


def _build():
    import concourse.bacc as bacc
    import concourse.mybir as mybir
    from concourse.tile import TileContext

    BF = mybir.dt.bfloat16
    F32 = mybir.dt.float32
    MULT = mybir.AluOpType.mult
    ADD = mybir.AluOpType.add

    nc = bacc.Bacc(trn_type="TRN2", num_devices=N_CORES)
    x_sh = nc.dram_tensor("x_sh", [128, CC, XR, XW], BF, kind="ExternalInput")
    u_sh = nc.dram_tensor("u_sh", [128, FM, P, CC, KH, 128], BF,
                          kind="ExternalInput")
    mk_sh = nc.dram_tensor("mk_sh", [128, HC * W], BF, kind="ExternalInput")
    b_sh = nc.dram_tensor("b_sh", [128, FM], F32, kind="ExternalInput")
    y_sh = nc.dram_tensor("y_sh", [FM, 128, HC, W], F32, kind="ExternalOutput")

    with TileContext(nc) as tc:
        with (
            tc.tile_pool(name="const", bufs=1) as cpool,
            tc.tile_pool(name="xin", bufs=4) as xpool,
            tc.tile_pool(name="vscr", bufs=2) as spool,
            tc.tile_pool(name="psum", bufs=2, space="PSUM") as ppool,
            tc.tile_pool(name="pwarm", bufs=1, space="PSUM") as wpool,
            tc.tile_pool(name="tmp", bufs=2) as tpool,
            tc.tile_pool(name="outp", bufs=2) as opool,
        ):
            # PE warmup while the first DMAs land (HAM pre-warm + head fill)
            scratch = cpool.tile([128, NPX], BF)
            nc.vector.memset(scratch[:], 0.0)
            dps = wpool.tile([128, 512], F32, name="dps", tag="pw")
            for _ in range(NWARM):
                nc.tensor.matmul(dps[:, :NPX], scratch[:, :128], scratch[:],
                                 start=True, stop=True)

            ut = cpool.tile([128, FM, P, CC, KH, 128], BF)
            mt = cpool.tile([128, HC * W], BF)
            bt = cpool.tile([128, FM], F32)
            vt = cpool.tile([128, CC, P, XR, T], BF)

            # U for fm0 (A-half points first), then mask/bias, then rest;
            # rides the ACT ring while x rides the SP ring.
            nc.scalar.dma_start(out=ut[:, 0, 0:3], in_=u_sh[:, 0, 0:3])
            nc.scalar.dma_start(out=ut[:, 0, 3:P], in_=u_sh[:, 0, 3:P])
            nc.scalar.dma_start(out=mt[:], in_=mk_sh[:])
            nc.scalar.dma_start(out=bt[:], in_=b_sh[:])
            for fm in range(1, FM):
                nc.scalar.dma_start(out=ut[:, fm], in_=u_sh[:, fm])

            # x DMAs: row-chunks, cc-interleaved, in first-use order
            xts = {}
            for ci, (r0, nr) in enumerate(CHUNKS):
                for cc in range(CC):
                    xt = xpool.tile([128, nr, XW], BF, name=f"x_{ci}_{cc}",
                                    tag="xt")
                    nc.sync.dma_start(out=xt[:], in_=x_sh[:, cc, r0:r0 + nr])
                    xts[(ci, cc)] = xt

            def d(xt, nr, j):
                # stride-4 slice: cols j, j+4, ..., j+4*(T-1)
                return xt[:, 0:nr, j:j + 4 * (T - 1) + 1:4]

            def transform(ci, cc):
                r0, nr = CHUNKS[ci]
                xt = xts[(ci, cc)]
                st = spool.tile([128, nr, T], F32, name=f"s_{ci}_{cc}",
                                tag="st", bufs=2)
                s2 = spool.tile([128, nr, T], F32, name=f"s2_{ci}_{cc}",
                                tag="st2", bufs=2)
                v = vt[:, cc, :, r0:r0 + nr]
                d0, d1, d2 = d(xt, nr, 0), d(xt, nr, 1), d(xt, nr, 2)
                d3, d4, d5 = d(xt, nr, 3), d(xt, nr, 4), d(xt, nr, 5)
                # V0 = 4 d0 - 5 d2 + d4
                nc.vector.scalar_tensor_tensor(st[:], d2, -5.0, d4, MULT, ADD)
                nc.vector.scalar_tensor_tensor(v[:, 0], d0, 4.0, st[:], MULT, ADD)
                # V1 = m1 + m2, V2 = m1 - m2;  m1 = -4 d2 + d4, m2 = -4 d1 + d3
                nc.vector.scalar_tensor_tensor(st[:], d2, -4.0, d4, MULT, ADD)
                nc.vector.scalar_tensor_tensor(s2[:], d1, -4.0, d3, MULT, ADD)
                nc.vector.tensor_add(v[:, 1], st[:], s2[:])
                nc.vector.tensor_sub(v[:, 2], st[:], s2[:])
                # V3 = m3 + 2 m4, V4 = m3 - 2 m4;  m3 = d4 - d2, m4 = d3 - d1
                nc.vector.tensor_sub(st[:], d4, d2)
                nc.vector.tensor_sub(s2[:], d3, d1)
                nc.vector.scalar_tensor_tensor(v[:, 3], s2[:], 2.0, st[:], MULT, ADD)
                nc.vector.scalar_tensor_tensor(v[:, 4], s2[:], -2.0, st[:], MULT, ADD)
                # V5 = 4 d1 - 5 d3 + d5
                nc.vector.scalar_tensor_tensor(st[:], d3, -5.0, d5, MULT, ADD)
                nc.vector.scalar_tensor_tensor(v[:, 5], d1, 4.0, st[:], MULT, ADD)

            def half_mms(fm, g, pt, ph):
                # 36 accumulating matmuls for points ph*3 .. ph*3+2
                for cc in range(CC):
                    for kh in range(KH):
                        for pj in range(3):
                            p = ph * 3 + pj
                            rhs = vt[:, cc, p, RG * g + kh:RG * g + kh + RG, :]
                            nc.tensor.matmul(
                                pt[:, pj, :NPX], ut[:, fm, p, cc, kh], rhs,
                                start=(cc == 0 and kh == 0),
                                stop=(cc == CC - 1 and kh == KH - 1),
                            )

            def group(fm, g):
                ptA = ppool.tile([128, 3, 512], F32, name=f"psA_{fm}_{g}",
                                 tag="ps")
                half_mms(fm, g, ptA, 0)
                tmp = tpool.tile([128, 8, NPX], F32, name=f"tm_{fm}_{g}",
                                 tag="tm")
                s, dd, yA0 = tmp[:, 0], tmp[:, 1], tmp[:, 2]
                pp, q, y3B = tmp[:, 3], tmp[:, 4], tmp[:, 5]
                cA, cB = tmp[:, 6], tmp[:, 7]
                m0, m1, m2 = ptA[:, 0, :NPX], ptA[:, 1, :NPX], ptA[:, 2, :NPX]
                # DVE may read only ONE operand from PSUM; stage m1/m3 into
                # SBUF via the (otherwise idle) ScalarE.
                nc.scalar.copy(cA, m1)
                nc.vector.tensor_add(s, cA, m2)
                nc.vector.tensor_sub(dd, cA, m2)
                nc.vector.tensor_add(yA0, m0, s)

                ptB = ppool.tile([128, 3, 512], F32, name=f"psB_{fm}_{g}",
                                 tag="ps")
                half_mms(fm, g, ptB, 1)
                m3, m4, m5 = ptB[:, 0, :NPX], ptB[:, 1, :NPX], ptB[:, 2, :NPX]
                nc.scalar.copy(cB, m3)
                nc.vector.tensor_add(pp, cB, m4)
                nc.vector.tensor_sub(q, cB, m4)
                nc.vector.scalar_tensor_tensor(y3B, q, 8.0, m5, MULT, ADD)

                yt = opool.tile([128, RG, W], F32, name=f"y_{fm}_{g}", tag="yt")

                def ypos(k):
                    return yt[:, :, k:k + 4 * (T - 1) + 1:4]

                nc.vector.tensor_add(ypos(0), yA0, pp)
                nc.vector.scalar_tensor_tensor(ypos(1), q, 2.0, dd, MULT, ADD)
                nc.vector.scalar_tensor_tensor(ypos(2), pp, 4.0, s, MULT, ADD)
                nc.vector.tensor_add(ypos(3), y3B, dd)

                yf = yt[:].rearrange("p r w -> p (r w)")
                nc.vector.tensor_mul(yf, yf, mt[:, RG * W * g:RG * W * (g + 1)])
                nc.scalar.activation(
                    yt[:], yt[:],
                    mybir.ActivationFunctionType.Identity,
                    bias=bt[:, fm:fm + 1],
                )
                nc.sync.dma_start(out=y_sh[fm, :, RG * g:RG * (g + 1)], in_=yt[:])

            # interleave DVE emission: transforms for chunks feeding early
            # groups first, drains in between so PSUM recycles promptly.
            for cc in range(CC):
                transform(0, cc)
            for cc in range(CC):
                transform(1, cc)
            group(0, 0)
            for cc in range(CC):
                transform(2, cc)
            group(0, 1)
            for cc in range(CC):
                transform(3, cc)
            group(0, 2)
            for fm in range(1, FM):
                for g in range(NG):
                    group(fm, g)

    nc.compile()
    return nc


def _wino_mats():
    BT = np.array([
        [4, 0, -5, 0, 1, 0],
        [0, -4, -4, 1, 1, 0],
        [0, 4, -4, -1, 1, 0],
        [0, -2, -1, 2, 1, 0],
        [0, 2, -1, -2, 1, 0],
        [0, 4, 0, -5, 0, 1]], dtype=np.float64)
    G = np.array([
        [1 / 4, 0, 0],
        [-1 / 6, -1 / 6, -1 / 6],
        [-1 / 6, 1 / 6, -1 / 6],
        [1 / 24, 1 / 12, 1 / 6],
        [1 / 24, -1 / 12, 1 / 6],
        [0, 0, 1]], dtype=np.float64)
    AT = np.array([
        [1, 1, 1, 1, 1, 0],
        [0, 1, -1, 2, -2, 0],
        [0, 1, 1, 4, 4, 0],
        [0, 1, -1, 8, -8, 1]], dtype=np.float64)
    return BT, G, AT


def _pack(x, w, b, mask):
    x = np.asarray(x, dtype=np.float32)
    w = np.asarray(w, dtype=np.float32)
    b = np.asarray(b, dtype=np.float32)
    mask = np.asarray(mask)

    xp = np.zeros((C, H + 2, W + 2), dtype=np.float32)
    xp[:, 1:-1, 1:-1] = x[0]
    xp = xp.astype(ml_dtypes.bfloat16)

    _, G, _ = _wino_mats()
    # U[p, f, c, kh] = sum_j G[p, j] * w[f, c, kh, j]
    u = np.einsum("pj,fckj->pfck", G, w.astype(np.float64)).astype(np.float32)
    # -> [c_local, fm, p, cc, kh, f_local]
    u = u.reshape(P, FM, 128, CC, 128, KH)
    u = u.transpose(4, 1, 0, 3, 5, 2)
    u = np.ascontiguousarray(u).astype(ml_dtypes.bfloat16)

    b_re = np.ascontiguousarray(b.reshape(FM, 128).T)

    mf = mask.astype(ml_dtypes.bfloat16)
    in_maps = []
    for k in range(N_CORES):
        xs = xp[:, HC * k:HC * k + XR, :]                     # [512, 26, 194]
        xs = np.ascontiguousarray(
            xs.reshape(CC, 128, XR, XW).transpose(1, 0, 2, 3))
        ms = mf[HC * k:HC * k + HC].reshape(1, HC * W)
        in_maps.append({
            "x_sh": xs,
            "u_sh": u,
            "mk_sh": np.ascontiguousarray(np.broadcast_to(ms, (128, HC * W))),
            "b_sh": b_re,
        })
    return in_maps


def _unpack(results):
    slabs = []
    for k in range(N_CORES):
        ys = results[k]["y_sh"]                               # [4, 128, 24, 192]
        slabs.append(ys.reshape(F, HC, W))
    out = np.concatenate(slabs, axis=1)                       # [512, 192, 192]
    return out[None].astype(np.float32)


def _run(inputs, **run_kwargs):
    from concourse.bass_utils import run_bass_kernel_spmd

    if "nc" not in _CACHE:
        _CACHE["nc"] = _build()
    nc = _CACHE["nc"]
    in_maps = _pack(inputs["x"], inputs["w"], inputs["b"], inputs["mask"])
    res = run_bass_kernel_spmd(nc, in_maps, core_ids=list(range(N_CORES)), **run_kwargs)
    return _unpack(res.results), res


def kernel(**inputs):
    out, _ = _run(inputs)
    return out
